# revision 1
# baseline (speedup 1.0000x reference)
"""GNN message-passing (HBS) kernel for 8 Trainium2 NeuronCores.

Sharding: core k owns src rows [k*6250, (k+1)*6250) (1D graph parallel).
Edges are bucketed per core by src range; within a core they are split in
two passes by dst range (so dma_gather's int16 indices cover the msg table)
and packed into degree-sorted node tiles:
  - node tile = 128 nodes (one per partition), tile width = max out-degree
    in the tile; each node's edges occupy its partition's slot columns.
  - per edge slot we gather the 512B table row [msg[dst] (64 f32) | alpha_dst]
    built on-device as msg = x @ W, alpha_dst = x @ (W a2).
  - e = LeakyReLU(alpha_src[src] + alpha_dst[dst]) with the per-node
    alpha_src as a per-partition ACT bias (computed from a per-node gather
    of x rows against W a1).
  - row_sum and the weighted feature sum reduce along each partition's
    slot columns (DVE), then per-node rows scatter-add into HBM scratch in
    natural node order; a final pass divides by row_sum and writes out.
All arithmetic runs on-device in f32; host work is integer index/layout prep.
"""

import numpy as np

# Problem constants (nn_HBS_28338194219185).
N_CELLS = 50000
C = 64
NEG_SLOPE = 0.2
N_CORES = 8
P = 128

ROWS = N_CELLS // N_CORES          # 6250 src rows per core
SPLIT = 32768                      # dst table split (int16 gather indices)
N_TILES = (ROWS + P - 1) // P      # 49 node tiles per pass
NODE_PAD = N_TILES * P             # 6272 node slots
SCRATCH_ROWS = 6400                # accumulation scratch rows (>= NODE_PAD)
CH_COLS = 48                       # max slot-columns per gather chunk
REC = 2 * C                        # table record width (128 f32 = 512B)

_cache = {}


def _wrap16(vals):
    """[n] int -> [128, n//16] int16 in the 16-wrapped, 8x-replicated layout
    used by dma_gather/dma_scatter_add index tensors."""
    vals = np.asarray(vals)
    n = len(vals)
    assert n % 16 == 0
    arr16 = vals.astype(np.int16).reshape(n // 16, 16).T
    return np.tile(arr16, (8, 1)).copy()


def _prep_core_pass(s_loc, d_rel, ev):
    """Per-(core, pass) degree-sorted node packing. Host-side int/layout prep."""
    deg = np.bincount(s_loc, minlength=ROWS)
    order = np.argsort(-deg, kind="stable")
    dego = np.concatenate([deg[order], np.zeros(NODE_PAD - ROWS, np.int64)])
    widths = dego.reshape(N_TILES, P).max(axis=1)

    es_order = np.argsort(s_loc, kind="stable")
    es = s_loc[es_order]
    starts = np.concatenate([[0], np.cumsum(deg)])
    rank = np.arange(len(es)) - starts[es]
    pos = np.empty(ROWS, np.int64)
    pos[order] = np.arange(ROWS)
    return dict(
        widths=widths,
        order=order,
        es_pos=pos[es],
        rank=rank,
        ed=d_rel[es_order],
        eev=ev[es_order],
    )


def _fill_slots(cp, col_base, c_tot):
    """Slot arrays for one (core, pass) given the shared width profile."""
    tile_of = cp["es_pos"] // P
    p_of = cp["es_pos"] % P
    col = col_base[tile_of] + cp["rank"]
    flat = col * P + p_of
    gv = np.zeros(c_tot * P, np.int64)
    evs = np.zeros(c_tot * P, np.float32)
    msk = np.zeros(c_tot * P, np.float32)
    gv[flat] = cp["ed"]
    evs[flat] = cp["eev"]
    msk[flat] = 1.0
    # scatter idx: real nodes -> natural local row, dummies -> scratch rows
    perm_sc = np.concatenate([cp["order"], np.arange(ROWS, NODE_PAD)])
    # x-gather idx: dummies -> row 0 (in-bounds junk; masked downstream)
    perm_x = np.concatenate([cp["order"], np.zeros(NODE_PAD - ROWS, np.int64)])
    evmask = np.concatenate(
        [evs.reshape(c_tot, P).T, msk.reshape(c_tot, P).T], axis=1
    ).astype(np.float32)  # [128, 2*c_tot]: ev cols then mask cols
    return dict(
        g_idx=_wrap16(gv),
        evmask=np.ascontiguousarray(evmask),
        nidx_sc=_wrap16(perm_sc),
        nidx_x=_wrap16(perm_x),
    )


def _make_chunks(widths):
    """Group tiles into gather chunks of <= CH_COLS slot-columns (a single
    wider tile gets its own chunk). Returns (tile_lo, tile_hi, col_lo, col_hi)."""
    chunks = []
    col_base = np.concatenate([[0], np.cumsum(widths)])
    t = 0
    n = len(widths)
    while t < n:
        if widths[t] == 0:
            t += 1
            continue
        t0 = t
        cols = widths[t]
        t += 1
        while t < n and cols + widths[t] <= CH_COLS:
            cols += widths[t]
            t += 1
        chunks.append((t0, t, int(col_base[t0]), int(col_base[t])))
    return chunks


def _host_prep(x_source, edge_index, edge_values):
    src = edge_index[0].astype(np.int64)
    dst = edge_index[1].astype(np.int64)
    ev = edge_values.astype(np.float32)
    core_of = src // ROWS

    passes = []
    for lo, hi in [(0, SPLIT), (SPLIT, N_CELLS)]:
        cores = []
        for k in range(N_CORES):
            m = core_of == k
            s = src[m] - k * ROWS
            d = dst[m]
            e = ev[m]
            pm = (d >= lo) & (d < hi)
            cores.append(_prep_core_pass(s[pm], d[pm] - lo, e[pm]))
        widths = np.max([c["widths"] for c in cores], axis=0)
        col_base = np.concatenate([[0], np.cumsum(widths)])
        c_tot = int(col_base[-1])
        for c in cores:
            c.update(_fill_slots(c, col_base, c_tot))
        passes.append(
            dict(
                cores=cores,
                widths=[int(w) for w in widths],
                col_base=[int(b) for b in col_base],
                c_tot=c_tot,
                chunks=_make_chunks(widths),
                base=lo,
                n_rows=(SPLIT if lo == 0 else N_CELLS - SPLIT),
            )
        )

    x = np.ascontiguousarray(x_source.astype(np.float32))
    in_maps = []
    for k in range(N_CORES):
        im = {
            "x_full": x,
            "x_slice": np.ascontiguousarray(x[k * ROWS : (k + 1) * ROWS]),
        }
        for pi, pp in enumerate(passes):
            cp = pp["cores"][k]
            im[f"g_idx{pi}"] = cp["g_idx"]
            im[f"evmask{pi}"] = cp["evmask"]
            im[f"nidx_sc{pi}"] = cp["nidx_sc"]
            im[f"nidx_x{pi}"] = cp["nidx_x"]
        in_maps.append(im)
    return passes, in_maps


def _build(passes):
    """Build the SPMD bass program. Structure depends only on the shared
    width profiles in `passes` (identical across cores)."""
    import os
    import concourse.bacc as bacc
    import concourse.tile as tile
    import concourse.mybir as mybir
    from concourse.masks import make_identity

    f32 = mybir.dt.float32
    i16 = mybir.dt.int16
    Alu = mybir.AluOpType
    Act = mybir.ActivationFunctionType

    nc = bacc.Bacc("TRN2", target_bir_lowering=False, debug=False,
                   num_devices=N_CORES)

    # ---- DRAM tensors ----
    x_full_d = nc.dram_tensor("x_full", [N_CELLS, C], f32, kind="ExternalInput")
    x_slice_d = nc.dram_tensor("x_slice", [ROWS, C], f32, kind="ExternalInput")
    g_idx_d, evmask_d, nidx_sc_d, nidx_x_d = [], [], [], []
    for pi, pp in enumerate(passes):
        ct = pp["c_tot"]
        g_idx_d.append(nc.dram_tensor(f"g_idx{pi}", [P, ct * 8], i16,
                                      kind="ExternalInput"))
        evmask_d.append(nc.dram_tensor(f"evmask{pi}", [P, 2 * ct], f32,
                                       kind="ExternalInput"))
        nidx_sc_d.append(nc.dram_tensor(f"nidx_sc{pi}", [P, NODE_PAD // 16],
                                        i16, kind="ExternalInput"))
        nidx_x_d.append(nc.dram_tensor(f"nidx_x{pi}", [P, NODE_PAD // 16],
                                       i16, kind="ExternalInput"))
    table_d = nc.dram_tensor("msg_table", [N_CELLS, REC], f32, kind="Internal")
    outsc_d = nc.dram_tensor("out_scratch", [SCRATCH_ROWS, C], f32,
                             kind="Internal")
    rssc_d = nc.dram_tensor("rs_scratch", [SCRATCH_ROWS, C], f32,
                            kind="Internal")
    out_d = nc.dram_tensor("out", [ROWS, C], f32, kind="ExternalOutput")

    W_d = nc.dram_tensor("w_mat", [C, C], f32, kind="ExternalInput")
    a1_d = nc.dram_tensor("a_vec1", [C], f32, kind="ExternalInput")
    a2_d = nc.dram_tensor("a_vec2", [C], f32, kind="ExternalInput")

    CHW = max(CH_COLS, *(w for pp in passes for w in pp["widths"]))
    ZCOLS = SCRATCH_ROWS * C // P  # 3200
    PADC = max(N_TILES * C, ZCOLS)

    with tile.TileContext(nc) as tc:
        with (
            tc.tile_pool(name="const", bufs=1) as const,
            tc.tile_pool(name="prep_ps", bufs=1, space="PSUM") as prep_ps,
            tc.tile_pool(name="mpsum", bufs=2, space="PSUM") as mpsum,
            tc.tile_pool(name="msgio", bufs=2) as msgio,
            tc.tile_pool(name="edges", bufs=2) as edges,
            tc.tile_pool(name="onceb", bufs=1) as onceb,
            tc.tile_pool(name="small", bufs=3) as small,
            tc.tile_pool(name="fin", bufs=2) as fin,
        ):
            # ---- constants / weight prep ----
            ident = const.tile([P, P], f32)
            make_identity(nc, ident[:])
            w_sb = const.tile([C, C], f32)
            nc.sync.dma_start(w_sb[:], W_d[:])
            a1_sb = const.tile([C, 1], f32)
            a2_sb = const.tile([C, 1], f32)
            nc.sync.dma_start(a1_sb[:], a1_d[:, None])
            nc.sync.dma_start(a2_sb[:], a2_d[:, None])

            wt_p = prep_ps.tile([C, C], f32)
            nc.tensor.transpose(wt_p[:], w_sb[:], ident[:C, :C])
            wt_sb = const.tile([C, C], f32)
            nc.vector.tensor_copy(wt_sb[:], wt_p[:])

            # Waug = [W | W@a2]; wa1 row-broadcast for alpha_src.
            waug = const.tile([C, C + 1], f32)
            nc.sync.dma_start(waug[:, :C], W_d[:])
            v2_p = prep_ps.tile([C, 1], f32)
            nc.tensor.matmul(v2_p[:], wt_sb[:], a2_sb[:], start=True, stop=True)
            nc.vector.tensor_copy(waug[:, C : C + 1], v2_p[:])
            v1_p = prep_ps.tile([C, 1], f32)
            nc.tensor.matmul(v1_p[:], wt_sb[:], a1_sb[:], start=True, stop=True)
            v1_sb = const.tile([C, 1], f32)
            nc.vector.tensor_copy(v1_sb[:], v1_p[:])
            row_p = prep_ps.tile([1, C], f32)
            nc.tensor.matmul(row_p[:], v1_sb[:], ident[:C, :C], start=True,
                             stop=True)
            row_sb = const.tile([1, C], f32)
            nc.vector.tensor_copy(row_sb[:], row_p[:])
            wa1b = const.tile([P, C], f32)
            nc.gpsimd.partition_broadcast(wa1b[:], row_sb[:1, :])

            # ---- zero the accumulation scratches (rs_pad doubles as the
            # zero source; it is re-zeroed per pass before reuse) ----
            rs_pad = onceb.tile([P, PADC], f32, tag="rspad")
            nc.vector.memset(rs_pad[:], 0.0)
            for scr in (outsc_d, rssc_d):
                v = scr[:].rearrange("(p a) c -> p (a c)", p=P)
                nc.sync.dma_start(v, rs_pad[:, :ZCOLS])

            _stop = os.environ.get("K_STOP", "")
            _order = ["prep", "table", "bias", "edges", "scatter", ""]
            _lvl = _order.index(_stop if _stop in _order else "")
            # ---- phase 1: msg table rows = [msg | alpha_dst | junk] ----
            groups = [2048] * (N_CELLS // 2048)
            left = N_CELLS - sum(groups)          # 848
            tail = left % 256                      # 80
            if left - tail:
                groups.append(left - tail)         # 768
            if _lvl < 1:
                groups, tail = [], 0
            r0 = 0
            for nrows in groups:
                xin = msgio.tile([P, 16 * C], f32, tag="xin")
                nc.sync.dma_start(
                    xin[:, : (nrows // P) * C].rearrange(
                        "p (a c) -> p a c", c=C
                    ),
                    x_full_d[r0 : r0 + nrows].rearrange(
                        "(a p) c -> p a c", p=P
                    ),
                )
                stage = msgio.tile([P, 16 * REC], f32, tag="stage")
                nc.gpsimd.memset(stage[:], 0.0)
                for j in range(nrows // P):
                    tp = mpsum.tile([P, P], f32, tag="tp")
                    nc.tensor.transpose(
                        tp[:C, :], xin[:, j * C : (j + 1) * C], ident[:]
                    )
                    xt = msgio.tile([P, P], f32, tag="xt")
                    nc.vector.tensor_copy(xt[:C, :], tp[:C, :])
                    mp = mpsum.tile([P, C + 1], f32, tag="mp")
                    nc.tensor.matmul(
                        mp[:], xt[:C, :], waug[:], start=True, stop=True
                    )
                    nc.vector.tensor_copy(
                        stage[:, j * REC : j * REC + C + 1], mp[:]
                    )
                nc.sync.dma_start(
                    table_d[r0 : r0 + nrows].rearrange(
                        "(a p) c -> p a c", p=P
                    ),
                    stage[:, : (nrows // P) * REC].rearrange(
                        "p (a c) -> p a c", c=REC
                    ),
                )
                r0 += nrows
            if tail:  # last 80 rows
                xin = msgio.tile([P, 16 * C], f32, tag="xin")
                nc.sync.dma_start(xin[:tail, :C], x_full_d[r0 : r0 + tail])
                tp = mpsum.tile([P, P], f32, tag="tp")
                nc.tensor.transpose(
                    tp[:C, :tail], xin[:tail, :C], ident[:tail, :tail]
                )
                xt = msgio.tile([P, P], f32, tag="xt")
                nc.vector.tensor_copy(xt[:C, :tail], tp[:C, :tail])
                mp = mpsum.tile([P, C + 1], f32, tag="mp")
                nc.tensor.matmul(
                    mp[:tail, :], xt[:C, :tail], waug[:], start=True, stop=True
                )
                stage = msgio.tile([P, 16 * REC], f32, tag="stage")
                nc.gpsimd.memset(stage[:tail, :REC], 0.0)
                nc.vector.tensor_copy(stage[:tail, : C + 1], mp[:tail, :])
                nc.sync.dma_start(
                    table_d[r0 : r0 + tail], stage[:tail, :REC]
                )

            # ---- per pass: alpha_src bias, edge chunks, scatters ----
            for pi, pp in enumerate(passes if _lvl >= 2 else []):
                ct = pp["c_tot"]
                widths = pp["widths"]
                col_base = pp["col_base"]

                nidx_x_sb = small.tile([P, NODE_PAD // 16], i16, tag="nix")
                nc.sync.dma_start(nidx_x_sb[:], nidx_x_d[pi][:])
                nidx_sc_sb = small.tile([P, NODE_PAD // 16], i16, tag="nis")
                nc.sync.dma_start(nidx_sc_sb[:], nidx_sc_d[pi][:])

                # alpha_src per (partition, tile) from permuted x rows
                # SWDGE ring fits ~64 descs/DMA engine: cap calls at
                # 1024 indices (8 record-tiles) each.
                xg = onceb.tile([P, N_TILES, C], f32, tag="xg")
                for k0 in range(0, N_TILES, 8):
                    k1 = min(k0 + 8, N_TILES)
                    nidx = (k1 - k0) * P
                    nc.gpsimd.dma_gather(
                        xg[:, k0:k1, :], x_slice_d[:],
                        nidx_x_sb[:, k0 * 8 : k1 * 8],
                        num_idxs=nidx, num_idxs_reg=nidx, elem_size=C,
                    )
                bias = small.tile([P, N_TILES], f32, tag="bias")
                ascr = onceb.tile([P, N_TILES, C], f32, tag="ascr")
                nc.vector.tensor_tensor(
                    out=ascr[:],
                    in0=xg[:],
                    in1=wa1b[:, None, :].broadcast_to([P, N_TILES, C]),
                    op=Alu.mult,
                )
                nc.vector.reduce_sum(
                    out=bias[:], in_=ascr[:], axis=mybir.AxisListType.X
                )

                if _lvl < 3:
                    continue
                out_parts = onceb.tile([P, N_TILES * C], f32, tag=f"op{pi}")
                rs_parts = small.tile([P, N_TILES], f32, tag=f"rp{pi}")
                nc.vector.memset(out_parts[:], 0.0)
                nc.vector.memset(rs_parts[:], 0.0)

                for (t0, t1, c0, c1) in pp["chunks"]:
                    cols = c1 - c0
                    gidx = small.tile([P, CHW * 8], i16, tag="gidx")
                    nc.sync.dma_start(
                        gidx[:, : cols * 8], g_idx_d[pi][:, c0 * 8 : c1 * 8]
                    )
                    evm = small.tile([P, 2 * CHW], f32, tag="evm")
                    nc.sync.dma_start(evm[:, :cols], evmask_d[pi][:, c0:c1])
                    nc.sync.dma_start(
                        evm[:, CHW : CHW + cols],
                        evmask_d[pi][:, ct + c0 : ct + c1],
                    )
                    gbuf = edges.tile([P, CHW, REC], f32, tag="gbuf")
                    for s0 in range(0, cols, 8):
                        s1 = min(s0 + 8, cols)
                        nidx = (s1 - s0) * P
                        nc.gpsimd.dma_gather(
                            gbuf[:, s0:s1, :],
                            table_d[pp["base"] : pp["base"] + pp["n_rows"], :],
                            gidx[:, s0 * 8 : s1 * 8],
                            num_idxs=nidx,
                            num_idxs_reg=nidx,
                            elem_size=REC,
                        )
                    for t in range(t0, t1):
                        w = widths[t]
                        if w == 0:
                            continue
                        cr0 = col_base[t] - c0
                        ad = gbuf[:, cr0 : cr0 + w, C : C + 1].rearrange(
                            "p c f -> p (c f)"
                        )
                        A_t = small.tile([P, CHW], f32, tag="A")
                        S_t = small.tile([P, CHW], f32, tag="S")
                        nc.scalar.activation(
                            A_t[:, :w], ad, Act.Identity,
                            bias=bias[:, t : t + 1], scale=1.0,
                        )
                        nc.scalar.activation(
                            S_t[:, :w], ad, Act.Sign,
                            bias=bias[:, t : t + 1], scale=1.0,
                        )
                        cf = small.tile([P, CHW], f32, tag="cf")
                        nc.vector.tensor_scalar(
                            cf[:, :w], S_t[:, :w],
                            (1.0 - NEG_SLOPE) / 2.0, (1.0 + NEG_SLOPE) / 2.0,
                            op0=Alu.mult, op1=Alu.add,
                        )
                        e_t = small.tile([P, CHW], f32, tag="e")
                        nc.vector.tensor_tensor(
                            out=e_t[:, :w], in0=A_t[:, :w], in1=cf[:, :w],
                            op=Alu.mult,
                        )
                        em = small.tile([P, CHW], f32, tag="em")
                        nc.vector.tensor_tensor(
                            out=em[:, :w],
                            in0=e_t[:, :w],
                            in1=evm[:, CHW + cr0 : CHW + cr0 + w],
                            op=Alu.mult,
                        )
                        nc.vector.reduce_sum(
                            out=rs_parts[:, t : t + 1],
                            in_=em[:, :w],
                            axis=mybir.AxisListType.X,
                        )
                        w_t = small.tile([P, CHW], f32, tag="w")
                        nc.vector.tensor_tensor(
                            out=w_t[:, :w], in0=e_t[:, :w],
                            in1=evm[:, cr0 : cr0 + w], op=Alu.mult,
                        )
                        tmp = edges.tile([P, C, CHW], f32, tag="tmp")
                        nc.vector.tensor_tensor(
                            out=tmp[:, :, :w],
                            in0=gbuf[:, cr0 : cr0 + w, 0:C].rearrange(
                                "p c f -> p f c"
                            ),
                            in1=w_t[:, None, :w].broadcast_to([P, C, w]),
                            op=Alu.mult,
                        )
                        nc.vector.reduce_sum(
                            out=out_parts[:, t * C : (t + 1) * C],
                            in_=tmp[:, :, :w],
                            axis=mybir.AxisListType.X,
                        )

                if _lvl < 4:
                    continue
                # scatter out_parts and rs_parts (unique per-node rows)
                nc.vector.memset(rs_pad[:], 0.0)
                nc.vector.tensor_copy(
                    rs_pad[:, : N_TILES * C].rearrange(
                        "p (t c) -> p t c", c=C
                    )[:, :, 0:1].rearrange("p t f -> p (t f)"),
                    rs_parts[:],
                )
                op_v = out_parts[:].rearrange("p (t c) -> p t c", c=C)
                rs_v = rs_pad[:, : N_TILES * C].rearrange(
                    "p (t c) -> p t c", c=C
                )
                for k0 in range(0, N_TILES, 8):
                    k1 = min(k0 + 8, N_TILES)
                    nidx = (k1 - k0) * P
                    nc.gpsimd.dma_scatter_add(
                        outsc_d[:],
                        op_v[:, k0:k1, :],
                        nidx_sc_sb[:, k0 * 8 : k1 * 8],
                        num_idxs=nidx,
                        num_idxs_reg=nidx,
                        elem_size=C,
                    )
                    nc.gpsimd.dma_scatter_add(
                        rssc_d[:],
                        rs_v[:, k0:k1, :],
                        nidx_sc_sb[:, k0 * 8 : k1 * 8],
                        num_idxs=nidx,
                        num_idxs_reg=nidx,
                        elem_size=C,
                    )

            # ---- finalize: out = out_unnorm / row_sum (0 where empty) ----
            FGROUP = 2048
            done = 0
            while done < (ROWS if _lvl >= 5 else 0):
                nrows = min(FGROUP, ROWS - done)
                if nrows >= P:
                    nrows -= nrows % P
                blocks = (nrows + P - 1) // P
                full = nrows % P == 0
                use_p = P if full else nrows
                ob = fin.tile([P, FGROUP // P * C], f32, tag="ob")
                rb = fin.tile([P, FGROUP // P * C], f32, tag="rb")
                if full:
                    nc.sync.dma_start(
                        ob[:, : blocks * C].rearrange("p (a c) -> p a c", c=C),
                        outsc_d[done : done + nrows].rearrange(
                            "(a p) c -> p a c", p=P
                        ),
                    )
                    nc.sync.dma_start(
                        rb[:, : blocks * C].rearrange("p (a c) -> p a c", c=C),
                        rssc_d[done : done + nrows].rearrange(
                            "(a p) c -> p a c", p=P
                        ),
                    )
                else:
                    nc.sync.dma_start(ob[:use_p, :C],
                                      outsc_d[done : done + nrows])
                    nc.sync.dma_start(rb[:use_p, :C],
                                      rssc_d[done : done + nrows])
                for b in range(blocks):
                    rs0 = rb[:use_p, b * C : b * C + 1]
                    z = fin.tile([P, 1], f32, tag="z")
                    nc.vector.tensor_scalar(
                        z[:use_p], rs0, 0.0, None, op0=Alu.is_equal
                    )
                    rsn = fin.tile([P, 1], f32, tag="rsn")
                    nc.vector.tensor_tensor(
                        out=rsn[:use_p], in0=rs0, in1=z[:use_p], op=Alu.add
                    )
                    rcp = fin.tile([P, 1], f32, tag="rcp")
                    nc.vector.reciprocal(rcp[:use_p], rsn[:use_p])
                    nc.vector.tensor_scalar(
                        ob[:use_p, b * C : (b + 1) * C],
                        ob[:use_p, b * C : (b + 1) * C],
                        rcp[:use_p],
                        None,
                        op0=Alu.mult,
                    )
                if full:
                    nc.sync.dma_start(
                        out_d[done : done + nrows].rearrange(
                            "(a p) c -> p a c", p=P
                        ),
                        ob[:, : blocks * C].rearrange("p (a c) -> p a c", c=C),
                    )
                else:
                    nc.sync.dma_start(out_d[done : done + nrows],
                                      ob[:use_p, :C])
                done += nrows

    nc.compile()
    return nc


def _get_program(passes):
    key = tuple(pp["c_tot"] for pp in passes) + tuple(
        w for pp in passes for w in pp["widths"]
    )
    if key not in _cache:
        _cache[key] = _build(passes)
    return _cache[key]


def kernel(x_source, edge_index, edge_values, W, a):
    import concourse.bass_utils as bass_utils

    passes, in_maps = _host_prep(
        np.asarray(x_source), np.asarray(edge_index), np.asarray(edge_values)
    )
    W_np = np.ascontiguousarray(np.asarray(W, np.float32))
    a_np = np.asarray(a, np.float32)
    for im in in_maps:
        im["w_mat"] = W_np
        im["a_vec1"] = np.ascontiguousarray(a_np[:C])
        im["a_vec2"] = np.ascontiguousarray(a_np[C:])

    nc = _get_program(passes)
    res = bass_utils.run_bass_kernel_spmd(
        nc, in_maps, core_ids=list(range(N_CORES))
    )
    return np.concatenate([res.results[k]["out"] for k in range(N_CORES)], 0)

